# revision 7
# baseline (speedup 1.0000x reference)
"""GAT (2-layer, 3-head) forward on 8 Trainium2 NeuronCores.

Math: with LeakyReLU slope ALPHA=1.0 the edge score e_ij = s1_i + s2_j is
linear, and s1_i cancels inside the row softmax.  The masked softmax over
j therefore reduces to column weights w_j = exp(s2_j - C) restricted to
adj, giving

    h'_i = (sum_j adj_ij * w_j * h_j) / (sum_j adj_ij * w_j)

i.e. one adjacency matmul against G = w*h plus a thin denominator matmul
against the w columns.  Both GAT layers take this form.

Precision: tolerance is 2e-2 so the adjacency matmuls run in fp8 e4m3
DoubleRow mode (2x effective PE rate: 256-deep contraction per pass) with
G scaled by 16 to lift small w*h products out of the subnormal range;
projections (x@W, xcat@Wo) run in single bf16.  End-to-end rel err ~3e-3.

Sharding: rows of h' (nodes) across 8 cores; each core holds fp8
lhsT-layout adjacency columns A^T[:, slab] and computes its 512-row slab.
Per-head fp8 G slabs are AllGathered so gathers pipeline against the
adjacency matmuls; head 0's gather is split in two halves so its L1
matmul starts sooner.  Layer 2 avoids a serial softmax-max collective by
building g2 with the core-local max and rescaling the gathered slabs by
exp(C_local - C_global) (the 8 local maxes travel in a tiny parallel
gather).

Schedule: x loads and the tiny s2 store/gather go first on uncontended
queues (vector issues the small critical-path DMAs) so the s2 AllGather
completes while the bulk W/adj/Wo loads stream in the background.
"""
import sys

sys.path.insert(0, "/opt/trn_rl_repo")

import math
import numpy as np
import ml_dtypes

import concourse.bass as bass
import concourse.bacc as bacc
import concourse.mybir as mybir
import concourse.bass_isa as bass_isa
import concourse.tile as tile
from concourse.bass_utils import run_bass_kernel_spmd

BF16 = ml_dtypes.bfloat16
F8 = ml_dtypes.float8_e4m3fn

N = 4096
F = 768
HID = 768
NH = 3
NCLS = 256
NCORES = 8
SLAB = N // NCORES          # 512 rows per core
NIT = SLAB // 128           # 4 i-tiles per core
NJT = N // 128              # 32 j-tiles
NQT = NJT // 2              # 16 j-tile pairs (DoubleRow)
NFT = F // 128              # 6 f-tiles
NCT = HID // 128            # 6 feature col-tiles of G per head
GH = NH * HID               # 2304 xcat columns
NOT = GH // 128             # 18 xcat col-tiles
G2C = NCLS + 1              # 257 = classes + w2 column
PAD2 = 264                  # G2 row padded to 8B
LOGC = math.log(16.0)       # fp8 scale for G / w columns (cancels in num/den)

AF = mybir.ActivationFunctionType
ALU = mybir.AluOpType
DR = mybir.MatmulPerfMode.DoubleRow


def build():
    dt = mybir.dt
    nc = bacc.Bacc(num_devices=NCORES)

    adjT_d = nc.dram_tensor("adjT", [N, SLAB], dt.float8e4, kind="ExternalInput")
    xT_d = nc.dram_tensor("xT", [F, SLAB], dt.bfloat16, kind="ExternalInput")
    U_d = nc.dram_tensor("U", [F, 8], dt.bfloat16, kind="ExternalInput")
    W_d = nc.dram_tensor("W", [NH, F, HID], dt.bfloat16, kind="ExternalInput")
    Wo_d = nc.dram_tensor("Wo", [GH, G2C], dt.bfloat16, kind="ExternalInput")
    out_d = nc.dram_tensor("out", [SLAB, NCLS], dt.float32, kind="ExternalOutput")

    # DRAM scratch + collective buffers
    s2s_d = nc.dram_tensor("s2s", [SLAB, 4], dt.float32)
    s2f_d = nc.dram_tensor("s2f", [N, 4], dt.float32, addr_space="Shared")
    gs0a = nc.dram_tensor("gs0a", [SLAB // 2, HID], dt.float8e4)
    gs0b = nc.dram_tensor("gs0b", [SLAB // 2, HID], dt.float8e4)
    gf0a = nc.dram_tensor("gf0a", [N // 2, HID], dt.float8e4, addr_space="Shared")
    gf0b = nc.dram_tensor("gf0b", [N // 2, HID], dt.float8e4, addr_space="Shared")
    gs12 = [nc.dram_tensor(f"gs{h}", [SLAB, HID], dt.float8e4) for h in (1, 2)]
    gf12 = [nc.dram_tensor(f"gf{h}", [N, HID], dt.float8e4, addr_space="Shared")
            for h in (1, 2)]
    c2s_d = nc.dram_tensor("c2s", [1], dt.float32)
    c2f_d = nc.dram_tensor("c2f", [NCORES], dt.float32, addr_space="Shared")
    g2_slab = nc.dram_tensor("g2_slab", [SLAB, PAD2], dt.float8e4)
    g2_full = nc.dram_tensor("g2_full", [N, PAD2], dt.float8e4, addr_space="Shared")

    dw_s = nc.dram_tensor("dw_s", [8], dt.float32)
    dw_f = nc.dram_tensor("dw_f", [8 * NCORES], dt.float32, addr_space="Shared")

    rg = [list(range(NCORES))]

    with tile.TileContext(nc) as tc:
      with (
          tc.tile_pool(name="adjt", bufs=NQT) as p_adjt,
          tc.tile_pool(name="xw", bufs=1) as p_xw,
          tc.tile_pool(name="small", bufs=1) as p_sm,
          tc.tile_pool(name="xct", bufs=1) as p_xct,
      ):
        # warm up the collective core: the first collective pays a ~50us
        # bootstrap, so fire a tiny no-dep gather immediately
        nc.gpsimd.collective_compute(
            "AllGather", ALU.bypass, replica_groups=rg,
            ins=[dw_s[:]], outs=[dw_f[:]])
        # ---------------- x + u loads, s2 chain, s2 gather ------------------
        xsb = []
        xT_t = xT_d.rearrange("(ft p) i -> ft p i", p=128)
        for ft in range(NFT):
            t = p_xw.tile([128, SLAB], dt.bfloat16, tag="x", name="x", bufs=NFT)
            nc.sync.dma_start(t[:], xT_t[ft])
            xsb.append(t)
        u = p_sm.tile([128, NFT, 8], dt.bfloat16, tag="u", name="u")
        nc.scalar.dma_start(u[:], U_d.rearrange("(ft p) c -> p ft c", p=128))

        s2loc = p_sm.tile([128, NIT, 4], dt.float32, tag="s2loc", name="s2loc")
        with tc.tile_pool(name="psS", bufs=2, space="PSUM") as ps_s:
            for it in range(NIT):
                ps = ps_s.tile([128, 8], dt.float32, tag="psS", name="psS")
                for ft in range(NFT):
                    nc.tensor.matmul(ps[:], xsb[ft][:, it * 128:(it + 1) * 128],
                                     u[:, ft, :],
                                     start=(ft == 0), stop=(ft == NFT - 1))
                nc.vector.tensor_copy(s2loc[:, it, 0:4], ps[:, 0:4])
                nc.gpsimd.dma_start(s2s_d[it * 128:(it + 1) * 128, :],
                                    s2loc[:, it, :])
        nc.gpsimd.collective_compute(
            "AllGather", ALU.bypass, replica_groups=rg,
            ins=[s2s_d[:]], outs=[s2f_d[:]])

        # ---------------- bulk loads (overlap the s2 gather) ----------------
        wsb = [[None] * NFT for _ in range(NH)]
        W_t = W_d.rearrange("h (ft p) o -> h ft p o", p=128)
        for h in range(NH):
            for ft in range(NFT):
                t = p_xw.tile([128, HID], dt.bfloat16, tag="w", name="w",
                              bufs=NH * NFT)
                eng = nc.sync if (h * NFT + ft) % 2 == 0 else nc.scalar
                eng.dma_start(t[:], W_t[h, ft])
                wsb[h][ft] = t
        adjt = []
        adjT_t = adjT_d.rearrange("(qt t p) i -> qt p t i", t=2, p=128)
        for q in range(NQT):
            t = p_adjt.tile([128, 2, SLAB], dt.float8e4, tag="adjt", name="adjt")
            eng = nc.sync if q % 2 == 0 else nc.scalar
            eng.dma_start(t[:], adjT_t[q])
            adjt.append(t)
        wo = []
        Wo_t = Wo_d.rearrange("(ot p) c -> ot p c", p=128)
        for ot in range(NOT):
            t = p_sm.tile([128, G2C], dt.bfloat16, tag="wo", name="wo", bufs=NOT)
            eng = nc.sync if ot % 2 == 0 else nc.scalar
            eng.dma_start(t[:], Wo_t[ot])
            wo.append(t)

        # ---------------- derive w from gathered s2 -------------------------
        sf = p_sm.tile([128, NJT, 4], dt.float32, tag="sf", name="sf")
        nc.gpsimd.dma_start(sf[:], s2f_d.rearrange("(jt p) c -> p jt c", p=128))
        negC3 = p_sm.tile([128, NH], dt.float32, tag="negC3", name="negC3")
        for h in range(NH):
            m1 = p_sm.tile([128, 1], dt.float32, tag="m1", name="m1", bufs=2)
            nc.vector.tensor_reduce(m1[:], sf[:, :, h],
                                    axis=mybir.AxisListType.X, op=ALU.max)
            m2 = p_sm.tile([128, 1], dt.float32, tag="m2", name="m2", bufs=2)
            nc.gpsimd.partition_all_reduce(m2[:], m1[:], channels=128,
                                           reduce_op=bass_isa.ReduceOp.max)
            nc.vector.tensor_scalar(negC3[:, h:h + 1], m2[:], -1.0, LOGC,
                                    ALU.mult, ALU.add)
        # fp8 denominator weights W3[j, h] = 16*exp(s2_j - C_h) for all j
        w3f = p_sm.tile([128, NJT, 16], dt.float32, tag="w3f", name="w3f")
        nc.vector.memset(w3f[:], 0.0)
        for h in range(NH):
            nc.scalar.activation(w3f[:, :, h], sf[:, :, h], AF.Exp,
                                 bias=negC3[:, h:h + 1])
        w3q = p_sm.tile([128, NJT, 16], dt.float8e4, tag="w3q", name="w3q")
        nc.vector.tensor_copy(w3q[:], w3f[:])
        # slab weights for scaling h into G
        w_sb = []
        for h in range(NH):
            w = p_sm.tile([128, NIT], dt.float32, tag="wexp", name="wexp",
                          bufs=NH)
            nc.scalar.activation(w[:], s2loc[:, :, h], AF.Exp,
                                 bias=negC3[:, h:h + 1])
            w_sb.append(w)

        # ---------------- per head: h = x@W, G = fp8(w*h), gather ----------
        with tc.tile_pool(name="psA", bufs=2, space="PSUM") as ps_a:
            for h in range(NH):
                for it in range(NIT):
                    ps = ps_a.tile([128, HID], dt.float32, tag="psA", name="psA")
                    for ft in range(NFT):
                        xh = xsb[ft][:, it * 128:(it + 1) * 128]
                        nc.tensor.matmul(ps[:, 0:512], xh, wsb[h][ft][:, 0:512],
                                         start=(ft == 0), stop=(ft == NFT - 1))
                        nc.tensor.matmul(ps[:, 512:HID], xh,
                                         wsb[h][ft][:, 512:HID],
                                         start=(ft == 0), stop=(ft == NFT - 1))
                    gq = p_sm.tile([128, HID], dt.float8e4, tag="gq",
                                   name="gq", bufs=3)
                    nc.vector.tensor_scalar_mul(gq[:], ps[:],
                                                w_sb[h][:, it:it + 1])
                    rows = slice((it % 2) * 128, (it % 2) * 128 + 128)
                    if h == 0:
                        dst = gs0a if it < 2 else gs0b
                        nc.gpsimd.dma_start(dst[rows, :], gq[:])
                    else:
                        nc.gpsimd.dma_start(
                            gs12[h - 1][it * 128:(it + 1) * 128, :], gq[:])
                    if h == 0 and it == 1:
                        nc.gpsimd.collective_compute(
                            "AllGather", ALU.bypass, replica_groups=rg,
                            ins=[gs0a[:]], outs=[gf0a[:]])
                    elif h == 0 and it == 3:
                        nc.gpsimd.collective_compute(
                            "AllGather", ALU.bypass, replica_groups=rg,
                            ins=[gs0b[:]], outs=[gf0b[:]])
                if h > 0:
                    nc.gpsimd.collective_compute(
                        "AllGather", ALU.bypass, replica_groups=rg,
                        ins=[gs12[h - 1][:]], outs=[gf12[h - 1][:]])

            # ------------- denominators: psd = W3^T @ A^T, reciprocal -------
            rbc = []
            with tc.tile_pool(name="psD", bufs=1, space="PSUM") as ps_d:
                psd = ps_d.tile([128, SLAB], dt.float32, tag="psD", name="psD")
                for q in range(NQT):
                    nc.tensor.matmul(psd[0:16, :], w3q[:, 2 * q:2 * q + 2, :],
                                     adjt[q][:], start=(q == 0),
                                     stop=(q == NQT - 1), perf_mode=DR)
                recip3 = p_sm.tile([NH, SLAB], dt.float32, tag="recip3",
                                   name="recip3")
                nc.vector.reciprocal(recip3[:], psd[0:NH, :])
                for h in range(NH):
                    rrow = p_sm.tile([1, SLAB], dt.float32, tag="rrow",
                                     name="rrow", bufs=2)
                    nc.gpsimd.dma_start(rrow[:], recip3[h:h + 1, :])
                    rb = p_sm.tile([128, SLAB], dt.float32, tag="rbc",
                                   name="rbc", bufs=NH)
                    nc.gpsimd.partition_broadcast(rb[:], rrow[:], channels=128)
                    rbc.append(rb)

            # ------------- L1 adjacency matmuls + elu epilogue --------------
            xc = []
            g0av = gf0a.rearrange("(b t p) c -> b p t c", t=2, p=128)
            g0bv = gf0b.rearrange("(b t p) c -> b p t c", t=2, p=128)
            with (
                tc.tile_pool(name="gst", bufs=24) as p_gst,
                tc.tile_pool(name="etmp", bufs=1) as p_et,
                tc.tile_pool(name="ps1", bufs=3, space="PSUM") as ps_1,
            ):
                for h in range(NH):
                    gt = [None] * NQT
                    if h == 0:
                        # even qt rows live in gf0a, odd qt rows in gf0b
                        qorder = [q for q in range(NQT) if q % 2 == 0] + \
                                 [q for q in range(NQT) if q % 2 == 1]
                        for i, q in enumerate(qorder):
                            g = p_gst.tile([128, 2, HID], dt.float8e4,
                                           tag="gst", name="gst")
                            src = g0av if q % 2 == 0 else g0bv
                            eng = nc.sync if i % 2 == 0 else nc.scalar
                            eng.dma_start(g[:], src[q // 2])
                            gt[q] = g
                    else:
                        qorder = list(range(NQT))
                        gv = gf12[h - 1].rearrange("(qt t p) c -> qt p t c",
                                                   t=2, p=128)
                        for q in qorder:
                            g = p_gst.tile([128, 2, HID], dt.float8e4,
                                           tag="gst", name="gst")
                            eng = nc.sync if q % 2 == 0 else nc.scalar
                            eng.dma_start(g[:], gv[q])
                            gt[q] = g
                    for lct in range(NCT):
                        ps = ps_1.tile([128, SLAB], dt.float32, tag="ps1",
                                       name="ps1")
                        for i, q in enumerate(qorder):
                            nc.tensor.matmul(
                                ps[:], gt[q][:, :, lct * 128:(lct + 1) * 128],
                                adjt[q][:], start=(i == 0), stop=(i == NQT - 1),
                                perf_mode=DR)
                        # xcatT tile = elu(numT / den) in bf16
                        z = p_et.tile([128, SLAB], dt.float32, tag="z",
                                      name="z", bufs=2)
                        nc.vector.tensor_tensor(z[:], ps[:], rbc[h][:], ALU.mult)
                        e = p_et.tile([128, SLAB], dt.float32, tag="e",
                                      name="e", bufs=2)
                        nc.scalar.activation(e[:], z[:], AF.Exp)
                        nc.vector.tensor_scalar(e[:], e[:], 1.0, -1.0,
                                                ALU.min, ALU.add)
                        th = p_xct.tile([128, SLAB], dt.bfloat16, tag="xcp",
                                        name="xcp", bufs=NOT)
                        nc.vector.scalar_tensor_tensor(th[:], z[:], 0.0, e[:],
                                                       ALU.max, ALU.add)
                        xc.append(th)

        # ---------------- layer 2 ------------------------------------------
        with (
            tc.tile_pool(name="l2a", bufs=1) as p_l2a,
            tc.tile_pool(name="psh2", bufs=4, space="PSUM") as ps_h2,
        ):
            ps2l = [ps_h2.tile([128, G2C], dt.float32, tag="psh2",
                               name="psh2") for _ in range(NIT)]
            for ot in range(NOT):
                for it in range(NIT):
                    nc.tensor.matmul(ps2l[it][:],
                                     xc[ot][:, it * 128:(it + 1) * 128],
                                     wo[ot][:],
                                     start=(ot == 0), stop=(ot == NOT - 1))
            s2p = p_l2a.tile([128, NIT], dt.float32, tag="s2p", name="s2p")
            h2_sb = []
            for it in range(NIT):
                h2 = p_l2a.tile([128, NCLS], dt.float32, tag="h2", name="h2",
                                bufs=NIT)
                nc.vector.tensor_copy(h2[:], ps2l[it][:, 0:NCLS])
                h2_sb.append(h2)
                nc.vector.tensor_copy(s2p[:, it:it + 1], ps2l[it][:, NCLS:G2C])
            # local max -> tiny parallel gather of the 8 per-core maxes
            sm1 = p_l2a.tile([128, 1], dt.float32, tag="sm1", name="sm1")
            nc.vector.tensor_reduce(sm1[:], s2p[:],
                                    axis=mybir.AxisListType.X, op=ALU.max)
            sm2 = p_l2a.tile([128, 1], dt.float32, tag="sm2", name="sm2")
            nc.gpsimd.partition_all_reduce(sm2[:], sm1[:], channels=128,
                                           reduce_op=bass_isa.ReduceOp.max)
            nc.gpsimd.dma_start(c2s_d[:].rearrange("(o a) -> o a", o=1),
                                sm2[0:1, 0:1])
            nc.gpsimd.collective_compute(
                "AllGather", ALU.bypass, replica_groups=rg,
                ins=[c2s_d[:]], outs=[c2f_d[:]])
            negC2 = p_l2a.tile([128, 1], dt.float32, tag="negC2", name="negC2")
            nc.vector.tensor_scalar(negC2[:], sm2[:], -1.0, LOGC,
                                    ALU.mult, ALU.add)
            w2all = p_l2a.tile([128, NIT], dt.float32, tag="w2all", name="w2all")
            nc.scalar.activation(w2all[:], s2p[:], AF.Exp, bias=negC2[:])
            for it in range(NIT):
                rows = slice(it * 128, (it + 1) * 128)
                g2q = p_l2a.tile([128, PAD2], dt.float8e4, tag="g2q",
                                 name="g2q", bufs=2)
                nc.vector.tensor_scalar_mul(g2q[:, 0:NCLS], h2_sb[it][:],
                                            w2all[:, it:it + 1])
                nc.vector.tensor_copy(g2q[:, NCLS:G2C], w2all[:, it:it + 1])
                nc.vector.memset(g2q[:, G2C:PAD2], 0.0)
                nc.gpsimd.dma_start(g2_slab[rows, :], g2q[:])
            nc.gpsimd.collective_compute(
                "AllGather", ALU.bypass, replica_groups=rg,
                ins=[g2_slab[:]], outs=[g2_full[:]])
            # rescale factors exp(C_local - C_global) per source slab
            cm = p_l2a.tile([1, NCORES], dt.float32, tag="cm", name="cm")
            nc.gpsimd.dma_start(cm[:], c2f_d[:].rearrange("(o a) -> o a", o=1))
            negCg = p_l2a.tile([1, 1], dt.float32, tag="negCg", name="negCg")
            nc.vector.tensor_reduce(negCg[:], cm[:],
                                    axis=mybir.AxisListType.X,
                                    op=ALU.max, negate=True)
            fr = p_l2a.tile([1, NCORES], dt.float32, tag="fr", name="fr")
            nc.scalar.activation(fr[:], cm[:], AF.Exp, bias=negCg[:])
            fbc = p_l2a.tile([128, NCORES], dt.float32, tag="fbc", name="fbc")
            nc.gpsimd.partition_broadcast(fbc[:], fr[:], channels=128)

            # L2 adjacency matmul + final epilogue
            with (
                tc.tile_pool(name="g2t", bufs=NQT) as p_g2t,
                tc.tile_pool(name="fin", bufs=1) as p_f,
                tc.tile_pool(name="ps2", bufs=4, space="PSUM") as ps_2,
            ):
                g2v = g2_full.rearrange("(qt t p) c -> qt p t c", t=2, p=128)
                g2t = []
                for q in range(NQT):
                    t = p_g2t.tile([128, 2, PAD2], dt.float8e4, tag="g2t",
                                   name="g2t")
                    eng = nc.sync if q % 2 == 0 else nc.scalar
                    eng.dma_start(t[:], g2v[q])
                    g2t.append(t)
                for q in range(NQT):
                    for tt in range(2):
                        c = (2 * q + tt) // 4
                        nc.vector.tensor_scalar_mul(g2t[q][:, tt, :],
                                                    g2t[q][:, tt, :],
                                                    fbc[:, c:c + 1])
                ps2 = [ps_2.tile([128, PAD2], dt.float32, tag="ps2", name="ps2")
                       for _ in range(NIT)]
                for it in range(NIT):
                    for q in range(NQT):
                        nc.tensor.matmul(
                            ps2[it][:],
                            adjt[q][:, :, it * 128:(it + 1) * 128],
                            g2t[q][:], start=(q == 0), stop=(q == NQT - 1),
                            perf_mode=DR)
                for it in range(NIT):
                    r2 = p_f.tile([128, 1], dt.float32, tag="r2", name="r2",
                                  bufs=2)
                    nc.vector.reciprocal(r2[:], ps2[it][:, NCLS:G2C])
                    z = p_f.tile([128, NCLS], dt.float32, tag="z2", name="z2",
                                 bufs=2)
                    nc.vector.tensor_scalar_mul(z[:], ps2[it][:, 0:NCLS], r2[:])
                    e = p_f.tile([128, NCLS], dt.float32, tag="e2", name="e2",
                                 bufs=2)
                    nc.scalar.activation(e[:], z[:], AF.Exp)
                    nc.vector.tensor_scalar(e[:], e[:], 1.0, -1.0,
                                            ALU.min, ALU.add)
                    o = p_f.tile([128, NCLS], dt.float32, tag="o2", name="o2",
                                 bufs=2)
                    nc.vector.scalar_tensor_tensor(o[:], z[:], 0.0, e[:],
                                                   ALU.max, ALU.add)
                    negm = p_f.tile([128, 1], dt.float32, tag="negm",
                                    name="negm", bufs=2)
                    nc.vector.tensor_reduce(negm[:], o[:],
                                            axis=mybir.AxisListType.X,
                                            op=ALU.max, negate=True)
                    t = p_f.tile([128, NCLS], dt.float32, tag="texp",
                                 name="texp", bufs=2)
                    nc.scalar.activation(t[:], o[:], AF.Exp, bias=negm[:])
                    ssum = p_f.tile([128, 1], dt.float32, tag="ssum",
                                    name="ssum", bufs=2)
                    nc.vector.tensor_reduce(ssum[:], t[:],
                                            axis=mybir.AxisListType.X,
                                            op=ALU.add)
                    lg = p_f.tile([128, 1], dt.float32, tag="lg", name="lg",
                                  bufs=2)
                    nc.scalar.activation(lg[:], ssum[:], AF.Ln)
                    fin = p_f.tile([128, NCLS], dt.float32, tag="fin",
                                   name="fin", bufs=2)
                    nc.vector.tensor_scalar(fin[:], o[:], negm[:], lg[:],
                                            ALU.add, ALU.subtract)
                    nc.sync.dma_start(out_d[it * 128:(it + 1) * 128, :], fin[:])

    nc.finalize()
    return nc


_CACHE = {}


def prepare_inputs(x, adj, W_heads, a_heads, W_out, a_out):
    """Shard + lay out the full inputs for the 8 cores."""
    x2 = np.asarray(x, np.float32)[0]          # [N, F]
    adj2 = np.asarray(adj)[0]                  # [N, N] int32
    W3 = np.asarray(W_heads, np.float32).reshape(NH, F, HID)
    a3 = np.asarray(a_heads, np.float32)       # [NH, 2*HID, 1]
    Wo = np.asarray(W_out, np.float32).reshape(GH, NCLS)
    ao = np.asarray(a_out, np.float32)         # [2*NCLS, 1]

    # fold the edge-score projections into the weights:
    #   s2 = x @ (W @ a2),   s2' = xcat @ (Wo @ ao2)
    u = np.einsum("hfo,ho->hf", W3.astype(np.float64),
                  a3[:, HID:, 0].astype(np.float64)).astype(np.float32)  # [NH,F]
    U = np.zeros((F, 8), BF16)
    for h in range(NH):
        U[:, h] = u[h].astype(BF16)
    u2 = (Wo.astype(np.float64) @ ao[NCLS:, 0].astype(np.float64)).astype(np.float32)
    Wo_ext = np.concatenate([Wo, u2[:, None]], axis=1).astype(BF16)  # [GH, 257]
    Wb = W3.astype(BF16)
    xT = np.ascontiguousarray(x2.T)            # [F, N]
    adjb = adj2.astype(F8)                     # exact 0/1

    in_maps = []
    for c in range(NCORES):
        sl = slice(c * SLAB, (c + 1) * SLAB)
        in_maps.append({
            "adjT": np.ascontiguousarray(adjb[sl, :].T),
            "xT": np.ascontiguousarray(xT[:, sl]).astype(BF16),
            "U": U,
            "W": Wb,
            "Wo": Wo_ext,
        })
    return in_maps


def kernel(x, adj, W_heads, a_heads, W_out, a_out):
    if "nc" not in _CACHE:
        # touch the devices once so any residual bad state from a previous
        # process surfaces (and clears) before the real run
        try:
            import jax
            jax.block_until_ready(jax.numpy.zeros(8))
        except Exception:
            pass
        _CACHE["nc"] = build()
    nc = _CACHE["nc"]
    in_maps = prepare_inputs(x, adj, W_heads, a_heads, W_out, a_out)
    res = run_bass_kernel_spmd(nc, in_maps, list(range(NCORES)))
    out = np.concatenate([res.results[c]["out"] for c in range(NCORES)], axis=0)
    return out.reshape(1, N, NCLS)


# revision 8
# speedup vs baseline: 1.0660x; 1.0660x over previous
"""GAT (2-layer, 3-head) forward on 8 Trainium2 NeuronCores.

Math: with LeakyReLU slope ALPHA=1.0 the edge score e_ij = s1_i + s2_j is
linear, and s1_i cancels inside the row softmax.  The masked softmax over
j therefore reduces to column weights w_j = exp(s2_j - C) restricted to
adj, giving

    h'_i = (sum_j adj_ij * w_j * h_j) / (sum_j adj_ij * w_j)

i.e. one adjacency matmul against G = w*h plus a thin denominator matmul
against the w columns.  Both GAT layers take this form.

Precision: tolerance is 2e-2 so the adjacency matmuls run in fp8 e4m3
DoubleRow mode (2x effective PE rate: 256-deep contraction per pass) with
G scaled by 16 to lift small w*h products out of the subnormal range;
projections (x@W, xcat@Wo) run in single bf16.  End-to-end rel err ~3e-3.

Sharding: rows of h' (nodes) across 8 cores; each core holds fp8
lhsT-layout adjacency columns A^T[:, slab] and computes its 512-row slab.
Per-head fp8 G slabs are AllGathered so gathers pipeline against the
adjacency matmuls; head 0's gather is split in two halves so its L1
matmul starts sooner.  Layer 2 avoids a serial softmax-max collective by
building g2 with the core-local max and rescaling the gathered slabs by
exp(C_local - C_global) (the 8 local maxes travel in a tiny parallel
gather).

Schedule: x loads and the tiny s2 store/gather go first on uncontended
queues (vector issues the small critical-path DMAs) so the s2 AllGather
completes while the bulk W/adj/Wo loads stream in the background.
"""
import sys

sys.path.insert(0, "/opt/trn_rl_repo")

import math
import numpy as np
import ml_dtypes

import concourse.bass as bass
import concourse.bacc as bacc
import concourse.mybir as mybir
import concourse.bass_isa as bass_isa
import concourse.tile as tile
from concourse.bass_utils import run_bass_kernel_spmd

BF16 = ml_dtypes.bfloat16
F8 = ml_dtypes.float8_e4m3fn

N = 4096
F = 768
HID = 768
NH = 3
NCLS = 256
NCORES = 8
SLAB = N // NCORES          # 512 rows per core
NIT = SLAB // 128           # 4 i-tiles per core
NJT = N // 128              # 32 j-tiles
NQT = NJT // 2              # 16 j-tile pairs (DoubleRow)
NFT = F // 128              # 6 f-tiles
NCT = HID // 128            # 6 feature col-tiles of G per head
GH = NH * HID               # 2304 xcat columns
NOT = GH // 128             # 18 xcat col-tiles
G2C = NCLS + 1              # 257 = classes + w2 column
PAD2 = 264                  # G2 row padded to 8B
LOGC = math.log(16.0)       # fp8 scale for G / w columns (cancels in num/den)

AF = mybir.ActivationFunctionType
ALU = mybir.AluOpType
DR = mybir.MatmulPerfMode.DoubleRow


def build():
    dt = mybir.dt
    nc = bacc.Bacc(num_devices=NCORES)

    adjT_d = nc.dram_tensor("adjT", [N, SLAB], dt.float8e4, kind="ExternalInput")
    xT_d = nc.dram_tensor("xT", [F, SLAB], dt.bfloat16, kind="ExternalInput")
    U_d = nc.dram_tensor("U", [F, 8], dt.bfloat16, kind="ExternalInput")
    W_d = nc.dram_tensor("W", [NH, F, HID], dt.bfloat16, kind="ExternalInput")
    Wo_d = nc.dram_tensor("Wo", [GH, G2C], dt.bfloat16, kind="ExternalInput")
    out_d = nc.dram_tensor("out", [SLAB, NCLS], dt.float32, kind="ExternalOutput")

    # DRAM scratch + collective buffers
    s2s_d = nc.dram_tensor("s2s", [SLAB, 4], dt.float32)
    s2f_d = nc.dram_tensor("s2f", [N, 4], dt.float32, addr_space="Shared")
    gs = [nc.dram_tensor(f"gs{h}", [SLAB, HID], dt.float8e4) for h in range(NH)]
    gf = [nc.dram_tensor(f"gf{h}", [N, HID], dt.float8e4, addr_space="Shared")
          for h in range(NH)]
    c2s_d = nc.dram_tensor("c2s", [1], dt.float32)
    c2f_d = nc.dram_tensor("c2f", [NCORES], dt.float32, addr_space="Shared")
    g2_slab = nc.dram_tensor("g2_slab", [SLAB, PAD2], dt.float8e4)
    g2_full = nc.dram_tensor("g2_full", [N, PAD2], dt.float8e4, addr_space="Shared")

    rg = [list(range(NCORES))]

    with tile.TileContext(nc) as tc:
      with (
          tc.tile_pool(name="adjt", bufs=NQT) as p_adjt,
          tc.tile_pool(name="xw", bufs=1) as p_xw,
          tc.tile_pool(name="small", bufs=1) as p_sm,
          tc.tile_pool(name="xct", bufs=1) as p_xct,
      ):
        # ---------------- x + u loads, s2 chain, s2 gather ------------------
        xsb = []
        xT_t = xT_d.rearrange("(ft p) i -> ft p i", p=128)
        for ft in range(NFT):
            t = p_xw.tile([128, SLAB], dt.bfloat16, tag="x", name="x", bufs=NFT)
            nc.sync.dma_start(t[:], xT_t[ft])
            xsb.append(t)
        u = p_sm.tile([128, NFT, 8], dt.bfloat16, tag="u", name="u")
        nc.scalar.dma_start(u[:], U_d.rearrange("(ft p) c -> p ft c", p=128))

        s2loc = p_sm.tile([128, NIT, 4], dt.float32, tag="s2loc", name="s2loc")
        with tc.tile_pool(name="psS", bufs=2, space="PSUM") as ps_s:
            for it in range(NIT):
                ps = ps_s.tile([128, 8], dt.float32, tag="psS", name="psS")
                for ft in range(NFT):
                    nc.tensor.matmul(ps[:], xsb[ft][:, it * 128:(it + 1) * 128],
                                     u[:, ft, :],
                                     start=(ft == 0), stop=(ft == NFT - 1))
                nc.vector.tensor_copy(s2loc[:, it, 0:4], ps[:, 0:4])
                nc.sync.dma_start(s2s_d[it * 128:(it + 1) * 128, :],
                                  s2loc[:, it, :])
        nc.gpsimd.collective_compute(
            "AllGather", ALU.bypass, replica_groups=rg,
            ins=[s2s_d[:]], outs=[s2f_d[:]])

        # ---------------- bulk loads (overlap the s2 gather) ----------------
        wsb = [[None] * NFT for _ in range(NH)]
        W_t = W_d.rearrange("h (ft p) o -> h ft p o", p=128)
        for h in range(NH):
            for ft in range(NFT):
                t = p_xw.tile([128, HID], dt.bfloat16, tag="w", name="w",
                              bufs=NH * NFT)
                nc.scalar.dma_start(t[:], W_t[h, ft])
                wsb[h][ft] = t
        adjt = []
        adjT_t = adjT_d.rearrange("(qt t p) i -> qt p t i", t=2, p=128)
        for q in range(NQT):
            t = p_adjt.tile([128, 2, SLAB], dt.float8e4, tag="adjt", name="adjt")
            nc.scalar.dma_start(t[:], adjT_t[q])
            adjt.append(t)
        wo = []
        Wo_t = Wo_d.rearrange("(ot p) c -> ot p c", p=128)
        for ot in range(NOT):
            t = p_sm.tile([128, G2C], dt.bfloat16, tag="wo", name="wo", bufs=NOT)
            nc.scalar.dma_start(t[:], Wo_t[ot])
            wo.append(t)

        # ---------------- derive w from gathered s2 -------------------------
        sf = p_sm.tile([128, NJT, 4], dt.float32, tag="sf", name="sf")
        nc.sync.dma_start(sf[:], s2f_d.rearrange("(jt p) c -> p jt c", p=128))
        negC3 = p_sm.tile([128, NH], dt.float32, tag="negC3", name="negC3")
        for h in range(NH):
            m1 = p_sm.tile([128, 1], dt.float32, tag="m1", name="m1", bufs=2)
            nc.vector.tensor_reduce(m1[:], sf[:, :, h],
                                    axis=mybir.AxisListType.X, op=ALU.max)
            m2 = p_sm.tile([128, 1], dt.float32, tag="m2", name="m2", bufs=2)
            nc.gpsimd.partition_all_reduce(m2[:], m1[:], channels=128,
                                           reduce_op=bass_isa.ReduceOp.max)
            nc.vector.tensor_scalar(negC3[:, h:h + 1], m2[:], -1.0, LOGC,
                                    ALU.mult, ALU.add)
        # fp8 denominator weights W3[j, h] = 16*exp(s2_j - C_h) for all j
        w3f = p_sm.tile([128, NJT, 16], dt.float32, tag="w3f", name="w3f")
        nc.vector.memset(w3f[:], 0.0)
        for h in range(NH):
            nc.scalar.activation(w3f[:, :, h], sf[:, :, h], AF.Exp,
                                 bias=negC3[:, h:h + 1])
        w3q = p_sm.tile([128, NJT, 16], dt.float8e4, tag="w3q", name="w3q")
        nc.vector.tensor_copy(w3q[:], w3f[:])
        # slab weights for scaling h into G
        w_sb = []
        for h in range(NH):
            w = p_sm.tile([128, NIT], dt.float32, tag="wexp", name="wexp",
                          bufs=NH)
            nc.scalar.activation(w[:], s2loc[:, :, h], AF.Exp,
                                 bias=negC3[:, h:h + 1])
            w_sb.append(w)

        # ---------------- per head: h = x@W, G = fp8(w*h), gather ----------
        with tc.tile_pool(name="psA", bufs=2, space="PSUM") as ps_a:
            for h in range(NH):
                for it in range(NIT):
                    ps = ps_a.tile([128, HID], dt.float32, tag="psA", name="psA")
                    for ft in range(NFT):
                        xh = xsb[ft][:, it * 128:(it + 1) * 128]
                        nc.tensor.matmul(ps[:, 0:512], xh, wsb[h][ft][:, 0:512],
                                         start=(ft == 0), stop=(ft == NFT - 1))
                        nc.tensor.matmul(ps[:, 512:HID], xh,
                                         wsb[h][ft][:, 512:HID],
                                         start=(ft == 0), stop=(ft == NFT - 1))
                    gq = p_sm.tile([128, HID], dt.float8e4, tag="gq",
                                   name="gq", bufs=3)
                    nc.vector.tensor_scalar_mul(gq[:], ps[:],
                                                w_sb[h][:, it:it + 1])
                    nc.sync.dma_start(gs[h][it * 128:(it + 1) * 128, :], gq[:])
                nc.gpsimd.collective_compute(
                    "AllGather", ALU.bypass, replica_groups=rg,
                    ins=[gs[h][:]], outs=[gf[h][:]])

            # ------------- denominators: psd = W3^T @ A^T, reciprocal -------
            rbc = []
            with tc.tile_pool(name="psD", bufs=1, space="PSUM") as ps_d:
                psd = ps_d.tile([128, SLAB], dt.float32, tag="psD", name="psD")
                for q in range(NQT):
                    nc.tensor.matmul(psd[0:16, :], w3q[:, 2 * q:2 * q + 2, :],
                                     adjt[q][:], start=(q == 0),
                                     stop=(q == NQT - 1), perf_mode=DR)
                recip3 = p_sm.tile([NH, SLAB], dt.float32, tag="recip3",
                                   name="recip3")
                nc.vector.reciprocal(recip3[:], psd[0:NH, :])
                for h in range(NH):
                    rrow = p_sm.tile([1, SLAB], dt.float32, tag="rrow",
                                     name="rrow", bufs=2)
                    nc.scalar.dma_start(rrow[:], recip3[h:h + 1, :])
                    rb = p_sm.tile([128, SLAB], dt.float32, tag="rbc",
                                   name="rbc", bufs=NH)
                    nc.gpsimd.partition_broadcast(rb[:], rrow[:], channels=128)
                    rbc.append(rb)

            # ------------- L1 adjacency matmuls + elu epilogue --------------
            xc = []
            with (
                tc.tile_pool(name="gst", bufs=24) as p_gst,
                tc.tile_pool(name="etmp", bufs=1) as p_et,
                tc.tile_pool(name="ps1", bufs=3, space="PSUM") as ps_1,
            ):
                for h in range(NH):
                    qorder = list(range(NQT))
                    gv = gf[h].rearrange("(qt t p) c -> qt p t c", t=2, p=128)
                    gt = []
                    for q in qorder:
                        g = p_gst.tile([128, 2, HID], dt.float8e4,
                                       tag="gst", name="gst")
                        nc.sync.dma_start(g[:], gv[q])
                        gt.append(g)
                    for lct in range(NCT):
                        ps = ps_1.tile([128, SLAB], dt.float32, tag="ps1",
                                       name="ps1")
                        for i, q in enumerate(qorder):
                            nc.tensor.matmul(
                                ps[:], gt[q][:, :, lct * 128:(lct + 1) * 128],
                                adjt[q][:], start=(i == 0), stop=(i == NQT - 1),
                                perf_mode=DR)
                        # xcatT tile = elu(numT / den) in bf16
                        z = p_et.tile([128, SLAB], dt.float32, tag="z",
                                      name="z", bufs=2)
                        nc.vector.tensor_tensor(z[:], ps[:], rbc[h][:], ALU.mult)
                        e = p_et.tile([128, SLAB], dt.float32, tag="e",
                                      name="e", bufs=2)
                        nc.scalar.activation(e[:], z[:], AF.Exp)
                        nc.vector.tensor_scalar(e[:], e[:], 1.0, -1.0,
                                                ALU.min, ALU.add)
                        th = p_xct.tile([128, SLAB], dt.bfloat16, tag="xcp",
                                        name="xcp", bufs=NOT)
                        nc.vector.scalar_tensor_tensor(th[:], z[:], 0.0, e[:],
                                                       ALU.max, ALU.add)
                        xc.append(th)

        # ---------------- layer 2 ------------------------------------------
        with (
            tc.tile_pool(name="l2a", bufs=1) as p_l2a,
            tc.tile_pool(name="psh2", bufs=4, space="PSUM") as ps_h2,
        ):
            ps2l = [ps_h2.tile([128, G2C], dt.float32, tag="psh2",
                               name="psh2") for _ in range(NIT)]
            for ot in range(NOT):
                for it in range(NIT):
                    nc.tensor.matmul(ps2l[it][:],
                                     xc[ot][:, it * 128:(it + 1) * 128],
                                     wo[ot][:],
                                     start=(ot == 0), stop=(ot == NOT - 1))
            s2p = p_l2a.tile([128, NIT], dt.float32, tag="s2p", name="s2p")
            h2_sb = []
            for it in range(NIT):
                h2 = p_l2a.tile([128, NCLS], dt.float32, tag="h2", name="h2",
                                bufs=NIT)
                nc.vector.tensor_copy(h2[:], ps2l[it][:, 0:NCLS])
                h2_sb.append(h2)
                nc.vector.tensor_copy(s2p[:, it:it + 1], ps2l[it][:, NCLS:G2C])
            # local max -> tiny parallel gather of the 8 per-core maxes
            sm1 = p_l2a.tile([128, 1], dt.float32, tag="sm1", name="sm1")
            nc.vector.tensor_reduce(sm1[:], s2p[:],
                                    axis=mybir.AxisListType.X, op=ALU.max)
            sm2 = p_l2a.tile([128, 1], dt.float32, tag="sm2", name="sm2")
            nc.gpsimd.partition_all_reduce(sm2[:], sm1[:], channels=128,
                                           reduce_op=bass_isa.ReduceOp.max)
            nc.sync.dma_start(c2s_d[:].rearrange("(o a) -> o a", o=1),
                              sm2[0:1, 0:1])
            nc.gpsimd.collective_compute(
                "AllGather", ALU.bypass, replica_groups=rg,
                ins=[c2s_d[:]], outs=[c2f_d[:]])
            negC2 = p_l2a.tile([128, 1], dt.float32, tag="negC2", name="negC2")
            nc.vector.tensor_scalar(negC2[:], sm2[:], -1.0, LOGC,
                                    ALU.mult, ALU.add)
            w2all = p_l2a.tile([128, NIT], dt.float32, tag="w2all", name="w2all")
            nc.scalar.activation(w2all[:], s2p[:], AF.Exp, bias=negC2[:])
            for it in range(NIT):
                rows = slice(it * 128, (it + 1) * 128)
                g2q = p_l2a.tile([128, PAD2], dt.float8e4, tag="g2q",
                                 name="g2q", bufs=2)
                nc.vector.tensor_scalar_mul(g2q[:, 0:NCLS], h2_sb[it][:],
                                            w2all[:, it:it + 1])
                nc.vector.tensor_copy(g2q[:, NCLS:G2C], w2all[:, it:it + 1])
                nc.vector.memset(g2q[:, G2C:PAD2], 0.0)
                nc.sync.dma_start(g2_slab[rows, :], g2q[:])
            nc.gpsimd.collective_compute(
                "AllGather", ALU.bypass, replica_groups=rg,
                ins=[g2_slab[:]], outs=[g2_full[:]])
            # rescale factors exp(C_local - C_global) per source slab
            cm = p_l2a.tile([1, NCORES], dt.float32, tag="cm", name="cm")
            nc.sync.dma_start(cm[:], c2f_d[:].rearrange("(o a) -> o a", o=1))
            negCg = p_l2a.tile([1, 1], dt.float32, tag="negCg", name="negCg")
            nc.vector.tensor_reduce(negCg[:], cm[:],
                                    axis=mybir.AxisListType.X,
                                    op=ALU.max, negate=True)
            fr = p_l2a.tile([1, NCORES], dt.float32, tag="fr", name="fr")
            nc.scalar.activation(fr[:], cm[:], AF.Exp, bias=negCg[:])
            fbc = p_l2a.tile([128, NCORES], dt.float32, tag="fbc", name="fbc")
            nc.gpsimd.partition_broadcast(fbc[:], fr[:], channels=128)

            # L2 adjacency matmul + final epilogue
            with (
                tc.tile_pool(name="g2t", bufs=NQT) as p_g2t,
                tc.tile_pool(name="fin", bufs=1) as p_f,
                tc.tile_pool(name="ps2", bufs=4, space="PSUM") as ps_2,
            ):
                g2v = g2_full.rearrange("(qt t p) c -> qt p t c", t=2, p=128)
                g2t = []
                for q in range(NQT):
                    t = p_g2t.tile([128, 2, PAD2], dt.float8e4, tag="g2t",
                                   name="g2t")
                    nc.sync.dma_start(t[:], g2v[q])
                    g2t.append(t)
                for q in range(NQT):
                    for tt in range(2):
                        c = (2 * q + tt) // 4
                        nc.vector.tensor_scalar_mul(g2t[q][:, tt, :],
                                                    g2t[q][:, tt, :],
                                                    fbc[:, c:c + 1])
                ps2 = [ps_2.tile([128, PAD2], dt.float32, tag="ps2", name="ps2")
                       for _ in range(NIT)]
                for it in range(NIT):
                    for q in range(NQT):
                        nc.tensor.matmul(
                            ps2[it][:],
                            adjt[q][:, :, it * 128:(it + 1) * 128],
                            g2t[q][:], start=(q == 0), stop=(q == NQT - 1),
                            perf_mode=DR)
                for it in range(NIT):
                    r2 = p_f.tile([128, 1], dt.float32, tag="r2", name="r2",
                                  bufs=2)
                    nc.vector.reciprocal(r2[:], ps2[it][:, NCLS:G2C])
                    z = p_f.tile([128, NCLS], dt.float32, tag="z2", name="z2",
                                 bufs=2)
                    nc.vector.tensor_scalar_mul(z[:], ps2[it][:, 0:NCLS], r2[:])
                    e = p_f.tile([128, NCLS], dt.float32, tag="e2", name="e2",
                                 bufs=2)
                    nc.scalar.activation(e[:], z[:], AF.Exp)
                    nc.vector.tensor_scalar(e[:], e[:], 1.0, -1.0,
                                            ALU.min, ALU.add)
                    o = p_f.tile([128, NCLS], dt.float32, tag="o2", name="o2",
                                 bufs=2)
                    nc.vector.scalar_tensor_tensor(o[:], z[:], 0.0, e[:],
                                                   ALU.max, ALU.add)
                    negm = p_f.tile([128, 1], dt.float32, tag="negm",
                                    name="negm", bufs=2)
                    nc.vector.tensor_reduce(negm[:], o[:],
                                            axis=mybir.AxisListType.X,
                                            op=ALU.max, negate=True)
                    t = p_f.tile([128, NCLS], dt.float32, tag="texp",
                                 name="texp", bufs=2)
                    nc.scalar.activation(t[:], o[:], AF.Exp, bias=negm[:])
                    ssum = p_f.tile([128, 1], dt.float32, tag="ssum",
                                    name="ssum", bufs=2)
                    nc.vector.tensor_reduce(ssum[:], t[:],
                                            axis=mybir.AxisListType.X,
                                            op=ALU.add)
                    lg = p_f.tile([128, 1], dt.float32, tag="lg", name="lg",
                                  bufs=2)
                    nc.scalar.activation(lg[:], ssum[:], AF.Ln)
                    fin = p_f.tile([128, NCLS], dt.float32, tag="fin",
                                   name="fin", bufs=2)
                    nc.vector.tensor_scalar(fin[:], o[:], negm[:], lg[:],
                                            ALU.add, ALU.subtract)
                    nc.sync.dma_start(out_d[it * 128:(it + 1) * 128, :], fin[:])

    nc.finalize()
    return nc


_CACHE = {}


def prepare_inputs(x, adj, W_heads, a_heads, W_out, a_out):
    """Shard + lay out the full inputs for the 8 cores."""
    x2 = np.asarray(x, np.float32)[0]          # [N, F]
    adj2 = np.asarray(adj)[0]                  # [N, N] int32
    W3 = np.asarray(W_heads, np.float32).reshape(NH, F, HID)
    a3 = np.asarray(a_heads, np.float32)       # [NH, 2*HID, 1]
    Wo = np.asarray(W_out, np.float32).reshape(GH, NCLS)
    ao = np.asarray(a_out, np.float32)         # [2*NCLS, 1]

    # fold the edge-score projections into the weights:
    #   s2 = x @ (W @ a2),   s2' = xcat @ (Wo @ ao2)
    u = np.einsum("hfo,ho->hf", W3.astype(np.float64),
                  a3[:, HID:, 0].astype(np.float64)).astype(np.float32)  # [NH,F]
    U = np.zeros((F, 8), BF16)
    for h in range(NH):
        U[:, h] = u[h].astype(BF16)
    u2 = (Wo.astype(np.float64) @ ao[NCLS:, 0].astype(np.float64)).astype(np.float32)
    Wo_ext = np.concatenate([Wo, u2[:, None]], axis=1).astype(BF16)  # [GH, 257]
    Wb = W3.astype(BF16)
    xT = np.ascontiguousarray(x2.T)            # [F, N]
    adjb = adj2.astype(F8)                     # exact 0/1

    in_maps = []
    for c in range(NCORES):
        sl = slice(c * SLAB, (c + 1) * SLAB)
        in_maps.append({
            "adjT": np.ascontiguousarray(adjb[sl, :].T),
            "xT": np.ascontiguousarray(xT[:, sl]).astype(BF16),
            "U": U,
            "W": Wb,
            "Wo": Wo_ext,
        })
    return in_maps


def kernel(x, adj, W_heads, a_heads, W_out, a_out):
    if "nc" not in _CACHE:
        # touch the devices once so any residual bad state from a previous
        # process surfaces (and clears) before the real run
        try:
            import jax
            jax.block_until_ready(jax.numpy.zeros(8))
        except Exception:
            pass
        _CACHE["nc"] = build()
    nc = _CACHE["nc"]
    in_maps = prepare_inputs(x, adj, W_heads, a_heads, W_out, a_out)
    res = run_bass_kernel_spmd(nc, in_maps, list(range(NCORES)))
    out = np.concatenate([res.results[c]["out"] for c in range(NCORES)], axis=0)
    return out.reshape(1, N, NCLS)


# revision 10
# speedup vs baseline: 1.1314x; 1.0614x over previous
"""GAT (2-layer, 3-head) forward on 8 Trainium2 NeuronCores.

Math: with LeakyReLU slope ALPHA=1.0 the edge score e_ij = s1_i + s2_j is
linear, and s1_i cancels inside the row softmax.  The masked softmax over
j therefore reduces to column weights w_j = exp(s2_j - C) restricted to
adj, giving

    h'_i = (sum_j adj_ij * w_j * h_j) / (sum_j adj_ij * w_j)

i.e. one adjacency matmul against G = w*h plus a thin denominator matmul
against the w columns.  Both GAT layers take this form.

Precision: tolerance is 2e-2 so the adjacency matmuls run in fp8 e4m3
DoubleRow mode (2x effective PE rate: 256-deep contraction per pass) with
G scaled by 16 to lift small w*h products out of the subnormal range;
projections (x@W, xcat@Wo) run in single bf16.  End-to-end rel err ~3e-3.

Sharding: rows of h' (nodes) across 8 cores; each core holds fp8
lhsT-layout adjacency columns A^T[:, slab] and computes its 512-row slab.
Per-head fp8 G slabs are AllGathered so gathers pipeline against the
adjacency matmuls; head 0's gather is split in two halves so its L1
matmul starts sooner.  Layer 2 avoids a serial softmax-max collective by
building g2 with the core-local max and rescaling the gathered slabs by
exp(C_local - C_global) (the 8 local maxes travel in a tiny parallel
gather).

Schedule: x loads and the tiny s2 store/gather go first on uncontended
queues (vector issues the small critical-path DMAs) so the s2 AllGather
completes while the bulk W/adj/Wo loads stream in the background.
"""
import sys

sys.path.insert(0, "/opt/trn_rl_repo")

import math
import numpy as np
import ml_dtypes

import concourse.bass as bass
import concourse.bacc as bacc
import concourse.mybir as mybir
import concourse.bass_isa as bass_isa
import concourse.tile as tile
from concourse.bass_utils import run_bass_kernel_spmd

BF16 = ml_dtypes.bfloat16
F8 = ml_dtypes.float8_e4m3fn

N = 4096
F = 768
HID = 768
NH = 3
NCLS = 256
NCORES = 8
SLAB = N // NCORES          # 512 rows per core
NIT = SLAB // 128           # 4 i-tiles per core
NJT = N // 128              # 32 j-tiles
NQT = NJT // 2              # 16 j-tile pairs (DoubleRow)
NFT = F // 128              # 6 f-tiles
NCT = HID // 128            # 6 feature col-tiles of G per head
GH = NH * HID               # 2304 xcat columns
NOT = GH // 128             # 18 xcat col-tiles
G2C = NCLS + 1              # 257 = classes + w2 column
PAD2 = 264                  # G2 row padded to 8B
LOGC = math.log(16.0)       # fp8 scale for G / w columns (cancels in num/den)

AF = mybir.ActivationFunctionType
ALU = mybir.AluOpType
DR = mybir.MatmulPerfMode.DoubleRow


def build():
    dt = mybir.dt
    nc = bacc.Bacc(num_devices=NCORES)

    adjT_d = nc.dram_tensor("adjT", [N, SLAB], dt.float8e4, kind="ExternalInput")
    xT_d = nc.dram_tensor("xT", [F, SLAB], dt.bfloat16, kind="ExternalInput")
    U_d = nc.dram_tensor("U", [F, 8], dt.bfloat16, kind="ExternalInput")
    W_d = nc.dram_tensor("W", [NH, F, HID], dt.bfloat16, kind="ExternalInput")
    Wo_d = nc.dram_tensor("Wo", [GH, G2C], dt.bfloat16, kind="ExternalInput")
    out_d = nc.dram_tensor("out", [SLAB, NCLS], dt.float32, kind="ExternalOutput")

    # DRAM scratch + collective buffers
    s2s_d = nc.dram_tensor("s2s", [SLAB, 4], dt.float32)
    s2f_d = nc.dram_tensor("s2f", [N, 4], dt.float32, addr_space="Shared")
    gs = [nc.dram_tensor(f"gs{h}", [SLAB, HID], dt.float8e4) for h in range(NH)]
    gf = [nc.dram_tensor(f"gf{h}", [N, HID], dt.float8e4, addr_space="Shared")
          for h in range(NH)]
    c2s_d = nc.dram_tensor("c2s", [1], dt.float32)
    c2f_d = nc.dram_tensor("c2f", [NCORES], dt.float32, addr_space="Shared")
    g2_slab = nc.dram_tensor("g2_slab", [SLAB, PAD2], dt.float8e4)
    g2_full = nc.dram_tensor("g2_full", [N, PAD2], dt.float8e4, addr_space="Shared")

    rg = [list(range(NCORES))]

    with tile.TileContext(nc) as tc:
      with (
          tc.tile_pool(name="adjt", bufs=NQT) as p_adjt,
          tc.tile_pool(name="xw", bufs=1) as p_xw,
          tc.tile_pool(name="small", bufs=1) as p_sm,
          tc.tile_pool(name="xct", bufs=1) as p_xct,
      ):
        # ---------------- x + u loads, s2 chain, s2 gather ------------------
        xsb = []
        xT_t = xT_d.rearrange("(ft p) i -> ft p i", p=128)
        for ft in range(NFT):
            t = p_xw.tile([128, SLAB], dt.bfloat16, tag="x", name="x", bufs=NFT)
            nc.sync.dma_start(t[:], xT_t[ft])
            xsb.append(t)
        u = p_sm.tile([128, NFT, 8], dt.bfloat16, tag="u", name="u")
        nc.scalar.dma_start(u[:], U_d.rearrange("(ft p) c -> p ft c", p=128))

        s2loc = p_sm.tile([128, NIT, 4], dt.float32, tag="s2loc", name="s2loc")
        with tc.tile_pool(name="psS", bufs=2, space="PSUM") as ps_s:
            for it in range(NIT):
                ps = ps_s.tile([128, 8], dt.float32, tag="psS", name="psS")
                for ft in range(NFT):
                    nc.tensor.matmul(ps[:], xsb[ft][:, it * 128:(it + 1) * 128],
                                     u[:, ft, :],
                                     start=(ft == 0), stop=(ft == NFT - 1))
                nc.vector.tensor_copy(s2loc[:, it, 0:4], ps[:, 0:4])
                nc.sync.dma_start(s2s_d[it * 128:(it + 1) * 128, :],
                                  s2loc[:, it, :])
        nc.gpsimd.collective_compute(
            "AllGather", ALU.bypass, replica_groups=rg,
            ins=[s2s_d[:]], outs=[s2f_d[:]])

        # ---------------- bulk loads (overlap the s2 gather) ----------------
        wsb = [[None] * NFT for _ in range(NH)]
        W_t = W_d.rearrange("h (ft p) o -> h ft p o", p=128)
        for h in range(NH):
            for ft in range(NFT):
                t = p_xw.tile([128, HID], dt.bfloat16, tag="w", name="w",
                              bufs=NH * NFT)
                nc.scalar.dma_start(t[:], W_t[h, ft])
                wsb[h][ft] = t
        adjt = []
        adjT_t = adjT_d.rearrange("(qt t p) i -> qt p t i", t=2, p=128)
        for q in range(NQT):
            t = p_adjt.tile([128, 2, SLAB], dt.float8e4, tag="adjt", name="adjt")
            nc.scalar.dma_start(t[:], adjT_t[q])
            adjt.append(t)
        wo = []
        Wo_t = Wo_d.rearrange("(ot p) c -> ot p c", p=128)
        for ot in range(NOT):
            t = p_sm.tile([128, G2C], dt.bfloat16, tag="wo", name="wo", bufs=NOT)
            nc.scalar.dma_start(t[:], Wo_t[ot])
            wo.append(t)

        # ---------------- derive w from gathered s2 -------------------------
        sf = p_sm.tile([128, NJT, 4], dt.float32, tag="sf", name="sf")
        nc.sync.dma_start(sf[:], s2f_d.rearrange("(jt p) c -> p jt c", p=128))
        negC3 = p_sm.tile([128, NH], dt.float32, tag="negC3", name="negC3")
        for h in range(NH):
            m1 = p_sm.tile([128, 1], dt.float32, tag="m1", name="m1", bufs=2)
            nc.vector.tensor_reduce(m1[:], sf[:, :, h],
                                    axis=mybir.AxisListType.X, op=ALU.max)
            m2 = p_sm.tile([128, 1], dt.float32, tag="m2", name="m2", bufs=2)
            nc.gpsimd.partition_all_reduce(m2[:], m1[:], channels=128,
                                           reduce_op=bass_isa.ReduceOp.max)
            nc.vector.tensor_scalar(negC3[:, h:h + 1], m2[:], -1.0, LOGC,
                                    ALU.mult, ALU.add)
        # fp8 denominator weights W3[j, h] = 16*exp(s2_j - C_h) for all j
        w3f = p_sm.tile([128, NJT, 16], dt.float32, tag="w3f", name="w3f")
        nc.vector.memset(w3f[:], 0.0)
        for h in range(NH):
            nc.scalar.activation(w3f[:, :, h], sf[:, :, h], AF.Exp,
                                 bias=negC3[:, h:h + 1])
        w3q = p_sm.tile([128, NJT, 16], dt.float8e4, tag="w3q", name="w3q")
        nc.vector.tensor_copy(w3q[:], w3f[:])
        # slab weights for scaling h into G
        w_sb = []
        for h in range(NH):
            w = p_sm.tile([128, NIT], dt.float32, tag="wexp", name="wexp",
                          bufs=NH)
            nc.scalar.activation(w[:], s2loc[:, :, h], AF.Exp,
                                 bias=negC3[:, h:h + 1])
            w_sb.append(w)

        # ---------------- per head: h = x@W, G = fp8(w*h), gather ----------
        with tc.tile_pool(name="psA", bufs=4, space="PSUM") as ps_a:
            for h in range(NH):
                for it in range(NIT):
                    ps = ps_a.tile([128, HID], dt.float32, tag="psA", name="psA")
                    for ft in range(NFT):
                        xh = xsb[ft][:, it * 128:(it + 1) * 128]
                        nc.tensor.matmul(ps[:, 0:512], xh, wsb[h][ft][:, 0:512],
                                         start=(ft == 0), stop=(ft == NFT - 1))
                        nc.tensor.matmul(ps[:, 512:HID], xh,
                                         wsb[h][ft][:, 512:HID],
                                         start=(ft == 0), stop=(ft == NFT - 1))
                    gq = p_sm.tile([128, HID], dt.float8e4, tag="gq",
                                   name="gq", bufs=3)
                    nc.vector.tensor_scalar_mul(gq[:], ps[:],
                                                w_sb[h][:, it:it + 1])
                    nc.sync.dma_start(gs[h][it * 128:(it + 1) * 128, :], gq[:])
                nc.gpsimd.collective_compute(
                    "AllGather", ALU.bypass, replica_groups=rg,
                    ins=[gs[h][:]], outs=[gf[h][:]])

        # ------------- denominators: psd = W3^T @ A^T, reciprocal -----------
        rbc = []
        with tc.tile_pool(name="psD", bufs=1, space="PSUM") as ps_d:
            psd = ps_d.tile([128, SLAB], dt.float32, tag="psD", name="psD")
            for q in range(NQT):
                nc.tensor.matmul(psd[0:16, :], w3q[:, 2 * q:2 * q + 2, :],
                                 adjt[q][:], start=(q == 0),
                                 stop=(q == NQT - 1), perf_mode=DR)
            recip3 = p_sm.tile([NH, SLAB], dt.float32, tag="recip3",
                               name="recip3")
            nc.vector.reciprocal(recip3[:], psd[0:NH, :])
            for h in range(NH):
                rrow = p_sm.tile([1, SLAB], dt.float32, tag="rrow",
                                 name="rrow", bufs=2)
                nc.scalar.dma_start(rrow[:], recip3[h:h + 1, :])
                rb = p_sm.tile([128, SLAB], dt.float32, tag="rbc",
                               name="rbc", bufs=NH)
                nc.gpsimd.partition_broadcast(rb[:], rrow[:], channels=128)
                rbc.append(rb)

        # ---- L1 adjacency matmuls + elu epilogue + incremental xcat@Wo -----
        xc = []
        with (
            tc.tile_pool(name="gst", bufs=24) as p_gst,
            tc.tile_pool(name="etmp", bufs=1) as p_et,
            tc.tile_pool(name="l2a", bufs=1) as p_l2a,
        ):
          with (
            tc.tile_pool(name="ps1", bufs=3, space="PSUM") as ps_1,
            tc.tile_pool(name="psh2", bufs=4, space="PSUM") as ps_h2,
          ):
            ps2l = [ps_h2.tile([128, G2C], dt.float32, tag="psh2",
                               name="psh2") for _ in range(NIT)]
            for h in range(NH):
                gv = gf[h].rearrange("(qt t p) c -> qt p t c", t=2, p=128)
                gt = []
                for q in range(NQT):
                    g = p_gst.tile([128, 2, HID], dt.float8e4,
                                   tag="gst", name="gst")
                    nc.sync.dma_start(g[:], gv[q])
                    gt.append(g)
                for lct in range(NCT):
                    ps = ps_1.tile([128, SLAB], dt.float32, tag="ps1",
                                   name="ps1")
                    for q in range(NQT):
                        nc.tensor.matmul(
                            ps[:], gt[q][:, :, lct * 128:(lct + 1) * 128],
                            adjt[q][:], start=(q == 0), stop=(q == NQT - 1),
                            perf_mode=DR)
                    # xcatT tile = elu(numT / den) in bf16
                    z = p_et.tile([128, SLAB], dt.float32, tag="z",
                                  name="z", bufs=2)
                    nc.vector.tensor_tensor(z[:], ps[:], rbc[h][:], ALU.mult)
                    e = p_et.tile([128, SLAB], dt.float32, tag="e",
                                  name="e", bufs=2)
                    nc.scalar.activation(e[:], z[:], AF.Exp)
                    nc.vector.tensor_scalar(e[:], e[:], 1.0, -1.0,
                                            ALU.min, ALU.add)
                    th = p_xct.tile([128, SLAB], dt.bfloat16, tag="xcp",
                                    name="xcp", bufs=NOT)
                    nc.vector.scalar_tensor_tensor(th[:], z[:], 0.0, e[:],
                                                   ALU.max, ALU.add)
                    xc.append(th)
                # fold this head's xcat tiles into layer 2 immediately
                for ot in range(h * NCT, (h + 1) * NCT):
                    for it in range(NIT):
                        nc.tensor.matmul(ps2l[it][:],
                                         xc[ot][:, it * 128:(it + 1) * 128],
                                         wo[ot][:],
                                         start=(ot == 0), stop=(ot == NOT - 1))

            # ---------------- layer 2 epilogue + masked softmax -------------
            s2p = p_l2a.tile([128, NIT], dt.float32, tag="s2p", name="s2p")
            h2_sb = []
            for it in range(NIT):
                h2 = p_l2a.tile([128, NCLS], dt.float32, tag="h2", name="h2",
                                bufs=NIT)
                nc.vector.tensor_copy(h2[:], ps2l[it][:, 0:NCLS])
                h2_sb.append(h2)
                nc.vector.tensor_copy(s2p[:, it:it + 1], ps2l[it][:, NCLS:G2C])
          # ps1/psh2 released; the tail below reuses those banks
          if True:
            # local max -> tiny parallel gather of the 8 per-core maxes
            sm1 = p_l2a.tile([128, 1], dt.float32, tag="sm1", name="sm1")
            nc.vector.tensor_reduce(sm1[:], s2p[:],
                                    axis=mybir.AxisListType.X, op=ALU.max)
            sm2 = p_l2a.tile([128, 1], dt.float32, tag="sm2", name="sm2")
            nc.gpsimd.partition_all_reduce(sm2[:], sm1[:], channels=128,
                                           reduce_op=bass_isa.ReduceOp.max)
            nc.sync.dma_start(c2s_d[:].rearrange("(o a) -> o a", o=1),
                              sm2[0:1, 0:1])
            nc.gpsimd.collective_compute(
                "AllGather", ALU.bypass, replica_groups=rg,
                ins=[c2s_d[:]], outs=[c2f_d[:]])
            negC2 = p_l2a.tile([128, 1], dt.float32, tag="negC2", name="negC2")
            nc.vector.tensor_scalar(negC2[:], sm2[:], -1.0, LOGC,
                                    ALU.mult, ALU.add)
            w2all = p_l2a.tile([128, NIT], dt.float32, tag="w2all", name="w2all")
            nc.scalar.activation(w2all[:], s2p[:], AF.Exp, bias=negC2[:])
            for it in range(NIT):
                rows = slice(it * 128, (it + 1) * 128)
                g2q = p_l2a.tile([128, PAD2], dt.float8e4, tag="g2q",
                                 name="g2q", bufs=2)
                nc.vector.tensor_scalar_mul(g2q[:, 0:NCLS], h2_sb[it][:],
                                            w2all[:, it:it + 1])
                nc.vector.tensor_copy(g2q[:, NCLS:G2C], w2all[:, it:it + 1])
                nc.vector.memset(g2q[:, G2C:PAD2], 0.0)
                nc.sync.dma_start(g2_slab[rows, :], g2q[:])
            nc.gpsimd.collective_compute(
                "AllGather", ALU.bypass, replica_groups=rg,
                ins=[g2_slab[:]], outs=[g2_full[:]])
            # rescale factors exp(C_local - C_global) per source slab
            cm = p_l2a.tile([1, NCORES], dt.float32, tag="cm", name="cm")
            nc.sync.dma_start(cm[:], c2f_d[:].rearrange("(o a) -> o a", o=1))
            negCg = p_l2a.tile([1, 1], dt.float32, tag="negCg", name="negCg")
            nc.vector.tensor_reduce(negCg[:], cm[:],
                                    axis=mybir.AxisListType.X,
                                    op=ALU.max, negate=True)
            fr = p_l2a.tile([1, NCORES], dt.float32, tag="fr", name="fr")
            nc.scalar.activation(fr[:], cm[:], AF.Exp, bias=negCg[:])
            fbc = p_l2a.tile([128, NCORES], dt.float32, tag="fbc", name="fbc")
            nc.gpsimd.partition_broadcast(fbc[:], fr[:], channels=128)

            # L2 adjacency matmul + final epilogue
            with (
                tc.tile_pool(name="g2t", bufs=NQT) as p_g2t,
                tc.tile_pool(name="fin", bufs=1) as p_f,
                tc.tile_pool(name="ps2", bufs=4, space="PSUM") as ps_2,
            ):
                g2v = g2_full.rearrange("(qt t p) c -> qt p t c", t=2, p=128)
                g2t = []
                for q in range(NQT):
                    t = p_g2t.tile([128, 2, PAD2], dt.float8e4, tag="g2t",
                                   name="g2t")
                    nc.sync.dma_start(t[:], g2v[q])
                    g2t.append(t)
                for q in range(NQT):
                    for tt in range(2):
                        c = (2 * q + tt) // 4
                        nc.vector.tensor_scalar_mul(g2t[q][:, tt, :],
                                                    g2t[q][:, tt, :],
                                                    fbc[:, c:c + 1])
                ps2 = [ps_2.tile([128, PAD2], dt.float32, tag="ps2", name="ps2")
                       for _ in range(NIT)]
                for it in range(NIT):
                    for q in range(NQT):
                        nc.tensor.matmul(
                            ps2[it][:],
                            adjt[q][:, :, it * 128:(it + 1) * 128],
                            g2t[q][:], start=(q == 0), stop=(q == NQT - 1),
                            perf_mode=DR)
                for it in range(NIT):
                    r2 = p_f.tile([128, 1], dt.float32, tag="r2", name="r2",
                                  bufs=2)
                    nc.vector.reciprocal(r2[:], ps2[it][:, NCLS:G2C])
                    z = p_f.tile([128, NCLS], dt.float32, tag="z2", name="z2",
                                 bufs=2)
                    nc.vector.tensor_scalar_mul(z[:], ps2[it][:, 0:NCLS], r2[:])
                    e = p_f.tile([128, NCLS], dt.float32, tag="e2", name="e2",
                                 bufs=2)
                    nc.scalar.activation(e[:], z[:], AF.Exp)
                    nc.vector.tensor_scalar(e[:], e[:], 1.0, -1.0,
                                            ALU.min, ALU.add)
                    o = p_f.tile([128, NCLS], dt.float32, tag="o2", name="o2",
                                 bufs=2)
                    nc.vector.scalar_tensor_tensor(o[:], z[:], 0.0, e[:],
                                                   ALU.max, ALU.add)
                    negm = p_f.tile([128, 1], dt.float32, tag="negm",
                                    name="negm", bufs=2)
                    nc.vector.tensor_reduce(negm[:], o[:],
                                            axis=mybir.AxisListType.X,
                                            op=ALU.max, negate=True)
                    t = p_f.tile([128, NCLS], dt.float32, tag="texp",
                                 name="texp", bufs=2)
                    nc.scalar.activation(t[:], o[:], AF.Exp, bias=negm[:])
                    ssum = p_f.tile([128, 1], dt.float32, tag="ssum",
                                    name="ssum", bufs=2)
                    nc.vector.tensor_reduce(ssum[:], t[:],
                                            axis=mybir.AxisListType.X,
                                            op=ALU.add)
                    lg = p_f.tile([128, 1], dt.float32, tag="lg", name="lg",
                                  bufs=2)
                    nc.scalar.activation(lg[:], ssum[:], AF.Ln)
                    fin = p_f.tile([128, NCLS], dt.float32, tag="fin",
                                   name="fin", bufs=2)
                    nc.vector.tensor_scalar(fin[:], o[:], negm[:], lg[:],
                                            ALU.add, ALU.subtract)
                    nc.sync.dma_start(out_d[it * 128:(it + 1) * 128, :], fin[:])

    nc.finalize()
    return nc


_CACHE = {}


def prepare_inputs(x, adj, W_heads, a_heads, W_out, a_out):
    """Shard + lay out the full inputs for the 8 cores."""
    x2 = np.asarray(x, np.float32)[0]          # [N, F]
    adj2 = np.asarray(adj)[0]                  # [N, N] int32
    W3 = np.asarray(W_heads, np.float32).reshape(NH, F, HID)
    a3 = np.asarray(a_heads, np.float32)       # [NH, 2*HID, 1]
    Wo = np.asarray(W_out, np.float32).reshape(GH, NCLS)
    ao = np.asarray(a_out, np.float32)         # [2*NCLS, 1]

    # fold the edge-score projections into the weights:
    #   s2 = x @ (W @ a2),   s2' = xcat @ (Wo @ ao2)
    u = np.einsum("hfo,ho->hf", W3.astype(np.float64),
                  a3[:, HID:, 0].astype(np.float64)).astype(np.float32)  # [NH,F]
    U = np.zeros((F, 8), BF16)
    for h in range(NH):
        U[:, h] = u[h].astype(BF16)
    u2 = (Wo.astype(np.float64) @ ao[NCLS:, 0].astype(np.float64)).astype(np.float32)
    Wo_ext = np.concatenate([Wo, u2[:, None]], axis=1).astype(BF16)  # [GH, 257]
    Wb = W3.astype(BF16)
    xT = np.ascontiguousarray(x2.T)            # [F, N]
    adjb = adj2.astype(F8)                     # exact 0/1

    in_maps = []
    for c in range(NCORES):
        sl = slice(c * SLAB, (c + 1) * SLAB)
        in_maps.append({
            "adjT": np.ascontiguousarray(adjb[sl, :].T),
            "xT": np.ascontiguousarray(xT[:, sl]).astype(BF16),
            "U": U,
            "W": Wb,
            "Wo": Wo_ext,
        })
    return in_maps


def kernel(x, adj, W_heads, a_heads, W_out, a_out):
    if "nc" not in _CACHE:
        # touch the devices once so any residual bad state from a previous
        # process surfaces (and clears) before the real run
        try:
            import jax
            jax.block_until_ready(jax.numpy.zeros(8))
        except Exception:
            pass
        _CACHE["nc"] = build()
    nc = _CACHE["nc"]
    in_maps = prepare_inputs(x, adj, W_heads, a_heads, W_out, a_out)
    res = run_bass_kernel_spmd(nc, in_maps, list(range(NCORES)))
    out = np.concatenate([res.results[c]["out"] for c in range(NCORES)], axis=0)
    return out.reshape(1, N, NCLS)
